# revision 14
# baseline (speedup 1.0000x reference)
"""CBOW negative-sampling loss kernel for Trainium2 (8 NeuronCores).

Data-parallel over batch (16384 -> 8 x 2048 rows). The two embedding
tables are concatenated into one fp8(e4m3) table [200000, 512B-stride]
(ctx rows 0..99999, center rows 100000..199999), pre-scaled by 2^13 so
the ~+-1.7e-3 weights sit in fp8's normal range; the 2^-26 descale and
the +-0.1 sign/mean fold into the final per-score multiplier. fp8 keeps
the loss error ~1e-7 (scores are ~1e-5 and the loss is 21*ln2 + O(x)).

Gathers use the custom InstDMAGatherAnt ("dma_gather"): ~1000 descriptors
per instruction sprayed over all 16 DMA engines, vs the old kernel's 496
generic indirect DMAs (128 descriptors each at ~1.3us fixed SWDGE cost,
~780us total). dma_gather takes int16 indices (<= 32767) and at most 1024
per instruction (Q7 scratch), so the table is addressed in 7 chunks of
32768 rows and the host buckets gather positions by chunk per superblock
(= 2 tiles = 256 batch rows x 31 slots = 7936 rows):

  stage 1: 10 chunk-pure gathers (HBM->SBUF; caps ~7 sigma over the
    multinomial per-chunk counts, dummy row-0 padding) land rows
    bucket-ordered in a [128, 75, 300] fp8 staging tile.
  stage 2: 4 SBUF-source gathers per tile un-permute 3968 rows into the
    canonical [128 rows, 31 slots, 300] fp8 tile (src token = idx&127 =
    partition, rank = idx>>7 = 300B stripe). Non-transpose SBUF source
    is not exposed in bass but the ucode supports it (bit-exact on HW),
    so instructions are built raw.

All gathers round-robin the 4 SWDGE queues (Q7 core pairs) so descriptor
generation runs 4-wide. Compute per tile on DVE reads fp8 directly:
strided reduce for the ctx sum (fp8->bf16), broadcast multiply (fp8 x
bf16 -> bf16) + reduce (->f32) for the 21 dots, per-slot +-0.1*2^-26
multiplier, then ACT exp + ln(1+e)-accumulate collapses the 21
log-sigmoid terms into acc[p, t] = per-row loss. Host sums acc [128, 16]
f32 per core and divides by 16384.
"""

import sys

for _p in ("/opt/trn_rl_repo", "/root/.axon_site/_ro/trn_rl_repo"):
    if _p not in sys.path:
        sys.path.append(_p)

import numpy as np
import ml_dtypes

VOCAB = 100000
D = 300
N_CTX = 10
N_NEG = 20
N_CN = 1 + N_NEG  # 21
N_SLOTS = 1 + N_CTX + N_NEG  # 31
N_CORES = 8
BATCH = 16384
P = 128
B_CORE = BATCH // N_CORES  # 2048
N_TILES = B_CORE // P  # 16
POS = N_SLOTS * P  # 3968 gather positions per tile

TILES_PER_SB = 2
SB_POS = TILES_PER_SB * POS  # 7936 positions per superblock

SCALE = 2.0**13
DESCALE = 0.1 / (SCALE * SCALE)

TBL_ROWS = 2 * VOCAB  # 200000
TBL_W = 512  # fp8 bytes -> 512B row stride (2*256B)
CHUNK = 32768
N_CHUNKS = -(-TBL_ROWS // CHUNK)  # 7
MAX_NI = 1024  # Q7 idx scratch limit per dma_gather

# Stage-1 instructions per superblock: (chunk, cap). Per-superblock chunk
# counts are multinomial: the 2560 ctx positions hit rows 0..99999
# (mean 839/sigma 24 in chunks 0-2), the 5376 cen/neg positions hit rows
# 100000..199999 (chunk 3 sees 1714+-35, chunks 4-5 1762+-34, chunk 6 only
# 3392 rows: 182+-13). Caps are ~7 sigma, split <= 1024 per instruction.
S1_INSTRS = [
    (0, 1008),
    (1, 1008),
    (2, 1008),
    (3, 992),
    (3, 976),
    (4, 1008),
    (4, 1008),
    (5, 1008),
    (5, 1008),
    (6, 288),
]
S1_IDX = sum(c for _, c in S1_INSTRS)  # 9312 indices sent per superblock
assert all(c % 16 == 0 and c <= MAX_NI for _, c in S1_INSTRS)
# Each instruction's destination region is 128-aligned (gathers write
# partition i%128 of sequential groups from their base rank).
S1_RANK0 = []
_r = 0
for _, _c in S1_INSTRS:
    S1_RANK0.append(_r)
    _r += -(-_c // P)
S1_RANKS = _r  # 75
# Stage-2: per tile, 4 gathers covering canonical groups [0,8), [8,16),
# [16,24), [24,31).
S2_INSTRS = [(0, 1024), (8, 1024), (16, 1024), (24, 896)]


def raw_dma_gather(
    nc,
    out_ap,
    in_ap,
    idxs_ap,
    num_idxs,
    elem_size,
    elem_step=None,
    queue_num=0,
    sbuf=None,
    single_packet=True,
    num_idxs_reg=None,
):
    """InstDMAGatherAnt builder; bass's dma_gather wrapper forbids elem
    sizes that are not 256B multiples and SBUF sources in non-transpose
    mode, but the ucode supports both (only the row *stride* is
    256B-quantized)."""
    from concourse import mybir

    assert num_idxs <= MAX_NI
    eng = nc.gpsimd
    if sbuf is None:
        ins_src = eng.lower_ap_dma(in_ap, for_custom_bir_dma=True)
        stride_bytes = elem_step * mybir.dt.size(in_ap.dtype)
        assert stride_bytes % 256 == 0 and stride_bytes // 256 < 256
        stride_256 = stride_bytes // 256
        sbuf_args = dict(
            sbuf_tokens_per_rank=0,
            sbuf_free_dim_per_rank=0,
            sbuf_free_dim_pad_per_rank=0,
            sbuf_byte_offset=0,
        )
    else:
        if in_ap.dtype != out_ap.dtype:
            in_ap = in_ap.bitcast(out_ap.dtype)
        ins_src = [eng.lower_ap(in_ap)]
        stride_256 = 0
        sbuf_args = sbuf
    return eng.add_instruction(
        mybir.InstDMAGatherAnt(
            name=eng.bass.get_next_instruction_name(),
            ins=[
                *ins_src,
                eng.lower_ap(idxs_ap),
                eng.lower_val_access(
                    eng.to_reg(num_idxs if num_idxs_reg is None else num_idxs_reg)
                ),
            ],
            outs=[eng.lower_ap(out_ap)],
            transpose=False,
            num_idxs=num_idxs,
            elem_size=elem_size,
            stride_bytes_256=stride_256,
            gen_mode=0,
            single_packet=single_packet,
            queue_num=queue_num,
            **sbuf_args,
        )
    )


def emit_cbow_body(nc, tc, tbl, idx1, idx2, cnt, signs, out, n_tiles, dbg=None):
    """Emit the per-core program body into an open TileContext.

    tbl:   [TBL_ROWS, TBL_W] fp8 DRAM (data in cols 0..299, x SCALE)
    idx1:  [P, n_sb * S1_IDX//16] int16 DRAM stage-1 planes
    idx2:  [P, n_tiles * POS//16] int16 DRAM stage-2 planes
    signs: [P, N_CN] f32 DRAM (rows [+DESCALE, -DESCALE x20])
    out:   [P, 1] f32 DRAM; out[p] = sum over this partition's rows/slots
           of ln(1+exp(-x)) (order-free: host just sums everything)
    cnt:   [P, n_sb * len(S1_INSTRS)] int32 DRAM true per-instruction
           index counts (stage-1 planes are padded with trailing -1,
           which the gather ucode trims to zero descriptors; the ring
           reservation in the decode comes from this runtime register)
    dbg:   optional {"canon": AP [P, n_tiles, N_SLOTS, D] fp8,
                     "scores": AP [P, n_tiles, N_CN] f32}
    """
    import contextlib

    from concourse import mybir

    assert n_tiles % TILES_PER_SB == 0
    n_sb = n_tiles // TILES_PER_SB
    f32 = mybir.dt.float32
    bf16 = mybir.dt.bfloat16
    fp8 = mybir.dt.float8e4
    i16 = mybir.dt.int16
    c1 = S1_IDX // 16  # stage-1 plane cols per superblock
    c2 = POS // 16  # stage-2 plane cols per tile
    qctr = [0]

    def next_q():
        q = qctr[0] % 4
        qctr[0] += 1
        return q

    with (
        tc.tile_pool(name="stage1", bufs=2) as s1pool,
        tc.tile_pool(name="canon", bufs=2) as cpool,
        tc.tile_pool(name="small", bufs=2) as spool,
        tc.tile_pool(name="accp", bufs=1) as apool,
    ):
        signs_sb = apool.tile([P, N_CN], f32)
        nc.sync.dma_start(out=signs_sb[:], in_=signs[:])
        n_i1 = len(S1_INSTRS)
        cnt_sb = apool.tile([P, n_sb * n_i1], mybir.dt.int32)
        nc.sync.dma_start(out=cnt_sb[:], in_=cnt[:])
        idx1_sb = apool.tile([P, n_sb * c1], i16)
        idx2_sb = apool.tile([P, n_tiles * c2], i16)

        def load_planes(sb):
            nc.sync.dma_start(
                out=idx1_sb[:, sb * c1 : (sb + 1) * c1],
                in_=idx1[:, sb * c1 : (sb + 1) * c1],
            )
            t0 = sb * TILES_PER_SB
            nc.sync.dma_start(
                out=idx2_sb[:, t0 * c2 : (t0 + TILES_PER_SB) * c2],
                in_=idx2[:, t0 * c2 : (t0 + TILES_PER_SB) * c2],
            )
        # scores for all tiles accumulate here; one Exp/Ln pass at the end
        # (the loss is a global sum, so slot/tile order is irrelevant)
        scores_all = apool.tile([P, n_tiles * N_CN], f32)
        warm = apool.tile([P, 1], f32)
        nc.vector.memset(warm[:], 0.0)
        nc.scalar.activation(
            out=warm[:], in_=warm[:],
            func=mybir.ActivationFunctionType.Exp, scale=-1.0,
        )
        nc.scalar.activation(
            out=warm[:], in_=warm[:],
            func=mybir.ActivationFunctionType.Ln, bias=1.0,
        )
        s1_tiles = {}

        est = tc if False else None  # placeholder
        rstack = contextlib.ExitStack()
        banks = [
            [
                rstack.enter_context(nc.gpsimd.register(name=f"cnt{b}_{k}"))
                for k in range(n_i1)
            ]
            for b in range(2)
        ]

        def emit_stage1(sb):
            bank = banks[sb % 2]
            nc.gpsimd.load(bank, cnt_sb[0:1, sb * n_i1 : (sb + 1) * n_i1])
            s1 = s1pool.tile([P, S1_RANKS, D], fp8, tag="s1")
            s1_tiles[sb] = s1
            off = 0
            for k, (c, cap) in enumerate(S1_INSTRS):
                raw_dma_gather(
                    nc,
                    s1[:, S1_RANK0[k] : S1_RANK0[k] + -(-cap // P), :],
                    tbl[c * CHUNK : min((c + 1) * CHUNK, TBL_ROWS), :D],
                    idx1_sb[:, sb * c1 + off // 16 : sb * c1 + (off + cap) // 16],
                    cap,
                    D,
                    TBL_W,
                    queue_num=next_q(),
                    num_idxs_reg=bank[k],
                )
                off += cap

        def emit_tile(sb, ti):
            t = sb * TILES_PER_SB + ti
            s1 = s1_tiles[sb]
            canon = cpool.tile([P, N_SLOTS, D], fp8, tag="canon")
            for g0, cap in S2_INSTRS:
                raw_dma_gather(
                    nc,
                    canon[:, g0 : g0 + -(-cap // P), :],
                    s1[:],
                    idx2_sb[:, t * c2 + g0 * 8 : t * c2 + g0 * 8 + cap // 16],
                    cap,
                    D,
                    None,
                    queue_num=next_q(),
                    sbuf=dict(
                        sbuf_tokens_per_rank=P,
                        sbuf_free_dim_per_rank=D,
                        sbuf_free_dim_pad_per_rank=0,
                        sbuf_byte_offset=0,
                    ),
                )
            if dbg is not None:
                nc.sync.dma_start(out=dbg["canon"][:, t, :, :], in_=canon[:])

            # ACT casts the whole tile once; DVE then runs 2x-rate bf16 ops
            cbf = cpool.tile([P, N_SLOTS, D], bf16, tag="cbf")
            nc.scalar.copy(out=cbf[:], in_=canon[:])

            # ctx_sum via contiguous-slab add tree (strided 1-elem reduce is
            # ~4x slower on DVE): 10 -> 5 -> (2,2,1) -> 1
            a5 = spool.tile([P, 5, D], bf16, tag="a5")
            nc.vector.tensor_tensor(
                out=a5[:], in0=cbf[:, 0:5, :], in1=cbf[:, 5:10, :],
                op=mybir.AluOpType.add,
            )
            b2 = spool.tile([P, 2, D], bf16, tag="b2")
            nc.vector.tensor_tensor(
                out=b2[:], in0=a5[:, 0:2, :], in1=a5[:, 2:4, :],
                op=mybir.AluOpType.add,
            )
            c1t = spool.tile([P, D], bf16, tag="c1t")
            nc.vector.tensor_tensor(
                out=c1t[:], in0=b2[:, 0, :], in1=b2[:, 1, :],
                op=mybir.AluOpType.add,
            )
            ctx_sum = spool.tile([P, D], bf16, tag="ctxsum")
            nc.vector.tensor_tensor(
                out=ctx_sum[:], in0=c1t[:], in1=a5[:, 4, :],
                op=mybir.AluOpType.add,
            )
            prod = spool.tile([P, N_CN, D], bf16, tag="prod")
            nc.vector.tensor_tensor(
                out=prod[:],
                in0=cbf[:, N_CTX:, :],
                in1=ctx_sum.unsqueeze(1).broadcast_to([P, N_CN, D]),
                op=mybir.AluOpType.mult,
            )
            sc = scores_all[:, t * N_CN : (t + 1) * N_CN]
            nc.vector.reduce_sum(out=sc, in_=prod[:], axis=mybir.AxisListType.X)
            # fold in the descale, the /10 ctx mean and the neg sign
            nc.vector.tensor_tensor(
                out=sc, in0=sc, in1=signs_sb[:], op=mybir.AluOpType.mult
            )
            if dbg is not None:
                nc.sync.dma_start(out=dbg["scores"][:, t, :], in_=sc)

        # software pipeline, skewed 1 superblock: superblock sb+1's HBM
        # gathers are queued before superblock sb's dependency-waiting
        # stage-2 work, but not so far ahead that ring-full awaits on
        # future work head-of-line block ready stage-2 decodes
        load_planes(0)
        emit_stage1(0)
        for sb in range(n_sb):
            if sb + 1 < n_sb:
                load_planes(sb + 1)
                emit_stage1(sb + 1)
            for ti in range(TILES_PER_SB):
                emit_tile(sb, ti)

        # one Exp + one Ln-accumulate over every tile's scores
        ex = apool.tile([P, n_tiles * N_CN], f32)
        lns = apool.tile([P, n_tiles * N_CN], f32)
        acc = apool.tile([P, 1], f32)
        nc.scalar.activation(
            out=ex[:],
            in_=scores_all[:],
            func=mybir.ActivationFunctionType.Exp,
            scale=-1.0,
        )
        nc.scalar.activation(
            out=lns[:],
            in_=ex[:],
            func=mybir.ActivationFunctionType.Ln,
            bias=1.0,
            accum_out=acc[:],
        )
        nc.sync.dma_start(out=out[:], in_=acc[:])
        rstack.close()


def build_program(n_tiles=N_TILES, n_cores=N_CORES, dbg=False):
    from concourse import mybir
    import concourse.bacc as bacc
    import concourse.tile as tile

    nc = bacc.Bacc(
        "TRN2",
        target_bir_lowering=False,
        debug=False,
        enable_asserts=False,
        num_devices=n_cores,
        num_swdge_queues=4,
    )
    n_sb = n_tiles // TILES_PER_SB
    tbl = nc.dram_tensor(
        "tbl", [TBL_ROWS, TBL_W], mybir.dt.float8e4, kind="ExternalInput"
    ).ap()
    idx1 = nc.dram_tensor(
        "idx1", [P, n_sb * S1_IDX // 16], mybir.dt.int16, kind="ExternalInput"
    ).ap()
    idx2 = nc.dram_tensor(
        "idx2", [P, n_tiles * POS // 16], mybir.dt.int16, kind="ExternalInput"
    ).ap()
    cnt = nc.dram_tensor(
        "cnt", [P, n_sb * len(S1_INSTRS)], mybir.dt.int32, kind="ExternalInput"
    ).ap()
    signs = nc.dram_tensor(
        "signs", [P, N_CN], mybir.dt.float32, kind="ExternalInput"
    ).ap()
    out = nc.dram_tensor(
        "out", [P, 1], mybir.dt.float32, kind="ExternalOutput"
    ).ap()
    dbg_aps = None
    if dbg:
        dbg_aps = {
            "canon": nc.dram_tensor(
                "canon_dbg",
                [P, n_tiles, N_SLOTS, D],
                mybir.dt.float8e4,
                kind="ExternalOutput",
            ).ap(),
            "scores": nc.dram_tensor(
                "scores_dbg", [P, n_tiles, N_CN], mybir.dt.float32,
                kind="ExternalOutput",
            ).ap(),
        }
    with tile.TileContext(nc) as tc:
        emit_cbow_body(nc, tc, tbl, idx1, idx2, cnt, signs, out, n_tiles, dbg=dbg_aps)
    nc.compile()
    return nc


_NC_CACHE = {}


def _get_program():
    if "nc" not in _NC_CACHE:
        _NC_CACHE["nc"] = build_program()
    return _NC_CACHE["nc"]


def make_table(context_weight, center_weight):
    tbl = np.zeros((TBL_ROWS, TBL_W), dtype=ml_dtypes.float8_e4m3fn)
    tbl[:VOCAB, :D] = (np.asarray(context_weight, np.float32) * SCALE).astype(
        ml_dtypes.float8_e4m3fn
    )
    tbl[VOCAB:, :D] = (np.asarray(center_weight, np.float32) * SCALE).astype(
        ml_dtypes.float8_e4m3fn
    )
    return tbl


def pack_gidx(context, center, negatives, batch=BATCH):
    """[batch, N_SLOTS] int32 global table rows: ctx cols 0..9, center col
    10 and negs 11..30 offset by VOCAB."""
    ctx = np.asarray(context, dtype=np.int64).reshape(batch, N_CTX)
    cen = np.asarray(center, dtype=np.int64).reshape(batch, 1) + VOCAB
    neg = np.asarray(negatives, dtype=np.int64).reshape(batch, N_NEG) + VOCAB
    return np.concatenate([ctx, cen, neg], axis=1).astype(np.int32)


def plane(lst):
    """int16 list (len%16==0) -> [128, len/16] plane: position i sits at
    partition i%16 (replicated across the 8 16-partition groups), col
    i//16 -- the layout the gather ucode's index reader expects."""
    arr = np.asarray(lst, dtype=np.int16).reshape(-1, 16).T  # [16, cols]
    return np.tile(arr, (8, 1))


def build_planes(gidx_core, n_tiles=N_TILES):
    """Stage-1/stage-2 index planes for one core.

    gidx_core: [n_tiles*P, N_SLOTS] int32 global table rows.
    Returns (idx1 [P, n_sb*S1_IDX/16], idx2 [P, n_tiles*POS/16]) int16.
    """
    n_sb = n_tiles // TILES_PER_SB
    p1 = np.empty((P, n_sb * S1_IDX // 16), dtype=np.int16)
    p2 = np.empty((P, n_tiles * POS // 16), dtype=np.int16)
    cnts = np.zeros((n_sb, len(S1_INSTRS)), dtype=np.int32)
    for sb in range(n_sb):
        rows = gidx_core[sb * TILES_PER_SB * P : (sb + 1) * TILES_PER_SB * P]
        # position q = tile_in_sb*POS + slot*128 + p
        pos_gidx = np.concatenate(
            [
                np.ascontiguousarray(rows[ti * P : (ti + 1) * P].T).reshape(-1)
                for ti in range(TILES_PER_SB)
            ]
        )  # [SB_POS]
        chunk = pos_gidx >> 15
        s1_idx = np.full(S1_IDX, -1, np.int16)
        s2_slot = np.empty(SB_POS, np.int32)
        filled = [0] * N_CHUNKS  # positions consumed per chunk
        off = 0
        for k, (c, cap) in enumerate(S1_INSTRS):
            sel = np.nonzero(chunk == c)[0][filled[c] : filled[c] + cap]
            filled[c] += sel.size
            n_c = sel.size
            s1_idx[off : off + n_c] = (pos_gidx[sel] & (CHUNK - 1)).astype(
                np.int16
            )
            if n_c == 0:  # keep >= 1 desc so ring accounting stays trivial
                s1_idx[off] = 0
                n_c = 1
            cnts[sb, k] = n_c
            s2_slot[sel] = S1_RANK0[k] * P + np.arange(sel.size)
            off += cap
        for c in range(N_CHUNKS):
            total = int((chunk == c).sum())
            if filled[c] != total:
                raise RuntimeError(
                    f"chunk {c}: {total} positions exceed caps ({filled[c]})"
                )
        p1[:, sb * (S1_IDX // 16) : (sb + 1) * (S1_IDX // 16)] = plane(s1_idx)
        s2 = s2_slot.astype(np.int16).reshape(TILES_PER_SB, POS)
        for ti in range(TILES_PER_SB):
            t = sb * TILES_PER_SB + ti
            p2[:, t * (POS // 16) : (t + 1) * (POS // 16)] = plane(s2[ti])
    cnt = np.tile(cnts.reshape(1, -1), (P, 1))
    return p1, p2, cnt


def make_signs():
    return np.tile(
        np.array([[DESCALE] + [-DESCALE] * N_NEG], dtype=np.float32), (P, 1)
    )


def make_in_maps(context, center, negatives, context_weight, center_weight):
    tbl = make_table(context_weight, center_weight)
    gidx = pack_gidx(context, center, negatives).reshape(N_CORES, B_CORE, N_SLOTS)
    signs = make_signs()
    maps = []
    for c in range(N_CORES):
        idx1, idx2, cnt = build_planes(gidx[c])
        maps.append(
            {"tbl": tbl, "idx1": idx1, "idx2": idx2, "cnt": cnt, "signs": signs}
        )
    return maps


def kernel(context, center, negatives, context_weight, center_weight):
    from concourse import bass_utils

    nc = _get_program()
    in_maps = make_in_maps(
        context, center, negatives, context_weight, center_weight
    )
    res = bass_utils.run_bass_kernel_spmd(nc, in_maps, core_ids=list(range(N_CORES)))
    acc = np.stack([r["out"] for r in res.results])  # [N_CORES, P, N_TILES]
    return np.array(acc.sum(dtype=np.float64) / BATCH, dtype=np.float32)


# revision 15
# speedup vs baseline: 1.0822x; 1.0822x over previous
"""CBOW negative-sampling loss kernel for Trainium2 (8 NeuronCores).

Data-parallel over batch (16384 -> 8 x 2048 rows). The two embedding
tables are concatenated into one fp8(e4m3) table [200000, 512B-stride]
(ctx rows 0..99999, center rows 100000..199999), pre-scaled by 2^13 so
the ~+-1.7e-3 weights sit in fp8's normal range; the 2^-26 descale and
the +-0.1 sign/mean fold into the final per-score multiplier. fp8 keeps
the loss error ~1e-7 (scores are ~1e-5 and the loss is 21*ln2 + O(x)).

Gathers use the custom InstDMAGatherAnt ("dma_gather"): ~1000 descriptors
per instruction sprayed over all 16 DMA engines, vs the old kernel's 496
generic indirect DMAs (128 descriptors each at ~1.3us fixed SWDGE cost,
~780us total). dma_gather takes int16 indices (<= 32767) and at most 1024
per instruction (Q7 scratch), so the table is addressed in 7 chunks of
32768 rows and the host buckets gather positions by chunk per superblock
(= 2 tiles = 256 batch rows x 31 slots = 7936 rows):

  stage 1: 10 chunk-pure gathers (HBM->SBUF; caps ~7 sigma over the
    multinomial per-chunk counts, dummy row-0 padding) land rows
    bucket-ordered in a [128, 75, 300] fp8 staging tile.
  stage 2: 4 SBUF-source gathers per tile un-permute 3968 rows into the
    canonical [128 rows, 31 slots, 300] fp8 tile (src token = idx&127 =
    partition, rank = idx>>7 = 300B stripe). Non-transpose SBUF source
    is not exposed in bass but the ucode supports it (bit-exact on HW),
    so instructions are built raw.

All gathers round-robin the 4 SWDGE queues (Q7 core pairs) so descriptor
generation runs 4-wide. Compute per tile on DVE reads fp8 directly:
strided reduce for the ctx sum (fp8->bf16), broadcast multiply (fp8 x
bf16 -> bf16) + reduce (->f32) for the 21 dots, per-slot +-0.1*2^-26
multiplier, then ACT exp + ln(1+e)-accumulate collapses the 21
log-sigmoid terms into acc[p, t] = per-row loss. Host sums acc [128, 16]
f32 per core and divides by 16384.
"""

import sys

for _p in ("/opt/trn_rl_repo", "/root/.axon_site/_ro/trn_rl_repo"):
    if _p not in sys.path:
        sys.path.append(_p)

import numpy as np
import ml_dtypes

VOCAB = 100000
D = 300
N_CTX = 10
N_NEG = 20
N_CN = 1 + N_NEG  # 21
N_SLOTS = 1 + N_CTX + N_NEG  # 31
N_CORES = 8
BATCH = 16384
P = 128
B_CORE = BATCH // N_CORES  # 2048
N_TILES = B_CORE // P  # 16
POS = N_SLOTS * P  # 3968 gather positions per tile

TILES_PER_SB = 2
SB_POS = TILES_PER_SB * POS  # 7936 positions per superblock

SCALE = 2.0**13
DESCALE = 0.1 / (SCALE * SCALE)

TBL_ROWS = 2 * VOCAB  # 200000
TBL_W = 512  # fp8 bytes -> 512B row stride (2*256B)
CHUNK = 32768
N_CHUNKS = -(-TBL_ROWS // CHUNK)  # 7
MAX_NI = 1024  # Q7 idx scratch limit per dma_gather

# Stage-1 instructions per superblock: (chunk, cap). Per-superblock chunk
# counts are multinomial: the 2560 ctx positions hit rows 0..99999
# (mean 839/sigma 24 in chunks 0-2), the 5376 cen/neg positions hit rows
# 100000..199999 (chunk 3 sees 1714+-35, chunks 4-5 1762+-34, chunk 6 only
# 3392 rows: 182+-13). Caps are ~7 sigma, split <= 1024 per instruction.
S1_INSTRS = [
    (0, 1008),
    (1, 1008),
    (2, 1008),
    (3, 992),
    (3, 976),
    (4, 1008),
    (4, 1008),
    (5, 1008),
    (5, 1008),
    (6, 288),
]
S1_IDX = sum(c for _, c in S1_INSTRS)  # 9312 indices sent per superblock
assert all(c % 16 == 0 and c <= MAX_NI for _, c in S1_INSTRS)
# Each instruction's destination region is 128-aligned (gathers write
# partition i%128 of sequential groups from their base rank).
S1_RANK0 = []
_r = 0
for _, _c in S1_INSTRS:
    S1_RANK0.append(_r)
    _r += -(-_c // P)
S1_RANKS = _r  # 75
# Stage-2: per tile, 4 gathers covering canonical groups [0,8), [8,16),
# [16,24), [24,31).
S2_INSTRS = [(0, 1024), (8, 1024), (16, 1024), (24, 896)]


def raw_dma_gather(
    nc,
    out_ap,
    in_ap,
    idxs_ap,
    num_idxs,
    elem_size,
    elem_step=None,
    queue_num=0,
    sbuf=None,
    single_packet=True,
):
    """InstDMAGatherAnt builder; bass's dma_gather wrapper forbids elem
    sizes that are not 256B multiples and SBUF sources in non-transpose
    mode, but the ucode supports both (only the row *stride* is
    256B-quantized)."""
    from concourse import mybir

    assert num_idxs <= MAX_NI
    eng = nc.gpsimd
    if sbuf is None:
        ins_src = eng.lower_ap_dma(in_ap, for_custom_bir_dma=True)
        stride_bytes = elem_step * mybir.dt.size(in_ap.dtype)
        assert stride_bytes % 256 == 0 and stride_bytes // 256 < 256
        stride_256 = stride_bytes // 256
        sbuf_args = dict(
            sbuf_tokens_per_rank=0,
            sbuf_free_dim_per_rank=0,
            sbuf_free_dim_pad_per_rank=0,
            sbuf_byte_offset=0,
        )
    else:
        if in_ap.dtype != out_ap.dtype:
            in_ap = in_ap.bitcast(out_ap.dtype)
        ins_src = [eng.lower_ap(in_ap)]
        stride_256 = 0
        sbuf_args = sbuf
    return eng.add_instruction(
        mybir.InstDMAGatherAnt(
            name=eng.bass.get_next_instruction_name(),
            ins=[
                *ins_src,
                eng.lower_ap(idxs_ap),
                eng.lower_val_access(eng.to_reg(num_idxs)),
            ],
            outs=[eng.lower_ap(out_ap)],
            transpose=False,
            num_idxs=num_idxs,
            elem_size=elem_size,
            stride_bytes_256=stride_256,
            gen_mode=0,
            single_packet=single_packet,
            queue_num=queue_num,
            **sbuf_args,
        )
    )


def emit_cbow_body(nc, tc, tbl, idx1, idx2, signs, out, n_tiles, dbg=None):
    """Emit the per-core program body into an open TileContext.

    tbl:   [TBL_ROWS, TBL_W] fp8 DRAM (data in cols 0..299, x SCALE)
    idx1:  [P, n_sb * S1_IDX//16] int16 DRAM stage-1 planes
    idx2:  [P, n_tiles * POS//16] int16 DRAM stage-2 planes
    signs: [P, N_CN] f32 DRAM (rows [+DESCALE, -DESCALE x20])
    out:   [P, 1] f32 DRAM; out[p] = sum over this partition's rows/slots
           of ln(1+exp(-x)) (order-free: host just sums everything)
    dbg:   optional {"canon": AP [P, n_tiles, N_SLOTS, D] fp8,
                     "scores": AP [P, n_tiles, N_CN] f32}
    """
    from concourse import mybir

    assert n_tiles % TILES_PER_SB == 0
    n_sb = n_tiles // TILES_PER_SB
    f32 = mybir.dt.float32
    bf16 = mybir.dt.bfloat16
    fp8 = mybir.dt.float8e4
    i16 = mybir.dt.int16
    c1 = S1_IDX // 16  # stage-1 plane cols per superblock
    c2 = POS // 16  # stage-2 plane cols per tile
    qctr = [0]

    def next_q():
        q = qctr[0] % 4
        qctr[0] += 1
        return q

    with (
        tc.tile_pool(name="stage1", bufs=2) as s1pool,
        tc.tile_pool(name="canon", bufs=2) as cpool,
        tc.tile_pool(name="small", bufs=2) as spool,
        tc.tile_pool(name="accp", bufs=1) as apool,
    ):
        signs_sb = apool.tile([P, N_CN], f32)
        nc.sync.dma_start(out=signs_sb[:], in_=signs[:])
        idx1_sb = apool.tile([P, n_sb * c1], i16)
        idx2_sb = apool.tile([P, n_tiles * c2], i16)

        def load_planes(sb):
            nc.sync.dma_start(
                out=idx1_sb[:, sb * c1 : (sb + 1) * c1],
                in_=idx1[:, sb * c1 : (sb + 1) * c1],
            )
            t0 = sb * TILES_PER_SB
            nc.sync.dma_start(
                out=idx2_sb[:, t0 * c2 : (t0 + TILES_PER_SB) * c2],
                in_=idx2[:, t0 * c2 : (t0 + TILES_PER_SB) * c2],
            )
        # scores for all tiles accumulate here; one Exp/Ln pass at the end
        # (the loss is a global sum, so slot/tile order is irrelevant)
        scores_all = apool.tile([P, n_tiles * N_CN], f32)
        warm = apool.tile([P, 1], f32)
        nc.vector.memset(warm[:], 0.0)
        nc.scalar.activation(
            out=warm[:], in_=warm[:],
            func=mybir.ActivationFunctionType.Exp, scale=-1.0,
        )
        nc.scalar.activation(
            out=warm[:], in_=warm[:],
            func=mybir.ActivationFunctionType.Ln, bias=1.0,
        )
        s1_tiles = {}

        def emit_stage1(sb):
            s1 = s1pool.tile([P, S1_RANKS, D], fp8, tag="s1")
            s1_tiles[sb] = s1
            off = 0
            for k, (c, cap) in enumerate(S1_INSTRS):
                raw_dma_gather(
                    nc,
                    s1[:, S1_RANK0[k] : S1_RANK0[k] + -(-cap // P), :],
                    tbl[c * CHUNK : min((c + 1) * CHUNK, TBL_ROWS), :D],
                    idx1_sb[:, sb * c1 + off // 16 : sb * c1 + (off + cap) // 16],
                    cap,
                    D,
                    TBL_W,
                    queue_num=next_q(),
                )
                off += cap

        def emit_tile(sb, ti):
            t = sb * TILES_PER_SB + ti
            s1 = s1_tiles[sb]
            canon = cpool.tile([P, N_SLOTS, D], fp8, tag="canon")
            for g0, cap in S2_INSTRS:
                raw_dma_gather(
                    nc,
                    canon[:, g0 : g0 + -(-cap // P), :],
                    s1[:],
                    idx2_sb[:, t * c2 + g0 * 8 : t * c2 + g0 * 8 + cap // 16],
                    cap,
                    D,
                    None,
                    queue_num=next_q(),
                    sbuf=dict(
                        sbuf_tokens_per_rank=P,
                        sbuf_free_dim_per_rank=D,
                        sbuf_free_dim_pad_per_rank=0,
                        sbuf_byte_offset=0,
                    ),
                )
            if dbg is not None:
                nc.sync.dma_start(out=dbg["canon"][:, t, :, :], in_=canon[:])

            # ACT casts the whole tile once; DVE then runs 2x-rate bf16 ops
            cbf = cpool.tile([P, N_SLOTS, D], bf16, tag="cbf")
            nc.scalar.copy(out=cbf[:], in_=canon[:])

            # ctx_sum via contiguous-slab add tree (strided 1-elem reduce is
            # ~4x slower on DVE): 10 -> 5 -> (2,2,1) -> 1
            a5 = spool.tile([P, 5, D], bf16, tag="a5")
            nc.vector.tensor_tensor(
                out=a5[:], in0=cbf[:, 0:5, :], in1=cbf[:, 5:10, :],
                op=mybir.AluOpType.add,
            )
            b2 = spool.tile([P, 2, D], bf16, tag="b2")
            nc.vector.tensor_tensor(
                out=b2[:], in0=a5[:, 0:2, :], in1=a5[:, 2:4, :],
                op=mybir.AluOpType.add,
            )
            c1t = spool.tile([P, D], bf16, tag="c1t")
            nc.vector.tensor_tensor(
                out=c1t[:], in0=b2[:, 0, :], in1=b2[:, 1, :],
                op=mybir.AluOpType.add,
            )
            ctx_sum = spool.tile([P, D], bf16, tag="ctxsum")
            nc.vector.tensor_tensor(
                out=ctx_sum[:], in0=c1t[:], in1=a5[:, 4, :],
                op=mybir.AluOpType.add,
            )
            prod = spool.tile([P, N_CN, D], bf16, tag="prod")
            nc.vector.tensor_tensor(
                out=prod[:],
                in0=cbf[:, N_CTX:, :],
                in1=ctx_sum.unsqueeze(1).broadcast_to([P, N_CN, D]),
                op=mybir.AluOpType.mult,
            )
            sc = scores_all[:, t * N_CN : (t + 1) * N_CN]
            nc.vector.reduce_sum(out=sc, in_=prod[:], axis=mybir.AxisListType.X)
            # fold in the descale, the /10 ctx mean and the neg sign
            nc.vector.tensor_tensor(
                out=sc, in0=sc, in1=signs_sb[:], op=mybir.AluOpType.mult
            )
            if dbg is not None:
                nc.sync.dma_start(out=dbg["scores"][:, t, :], in_=sc)

        # software pipeline, skewed 1 superblock: superblock sb+1's HBM
        # gathers are queued before superblock sb's dependency-waiting
        # stage-2 work, but not so far ahead that ring-full awaits on
        # future work head-of-line block ready stage-2 decodes
        load_planes(0)
        emit_stage1(0)
        for sb in range(n_sb):
            if sb + 1 < n_sb:
                load_planes(sb + 1)
                emit_stage1(sb + 1)
            for ti in range(TILES_PER_SB):
                emit_tile(sb, ti)

        # one Exp + one Ln-accumulate over every tile's scores
        ex = apool.tile([P, n_tiles * N_CN], f32)
        lns = apool.tile([P, n_tiles * N_CN], f32)
        acc = apool.tile([P, 1], f32)
        nc.scalar.activation(
            out=ex[:],
            in_=scores_all[:],
            func=mybir.ActivationFunctionType.Exp,
            scale=-1.0,
        )
        nc.scalar.activation(
            out=lns[:],
            in_=ex[:],
            func=mybir.ActivationFunctionType.Ln,
            bias=1.0,
            accum_out=acc[:],
        )
        nc.sync.dma_start(out=out[:], in_=acc[:])


def build_program(n_tiles=N_TILES, n_cores=N_CORES, dbg=False):
    from concourse import mybir
    import concourse.bacc as bacc
    import concourse.tile as tile

    nc = bacc.Bacc(
        "TRN2",
        target_bir_lowering=False,
        debug=False,
        enable_asserts=False,
        num_devices=n_cores,
        num_swdge_queues=4,
    )
    n_sb = n_tiles // TILES_PER_SB
    tbl = nc.dram_tensor(
        "tbl", [TBL_ROWS, TBL_W], mybir.dt.float8e4, kind="ExternalInput"
    ).ap()
    idx1 = nc.dram_tensor(
        "idx1", [P, n_sb * S1_IDX // 16], mybir.dt.int16, kind="ExternalInput"
    ).ap()
    idx2 = nc.dram_tensor(
        "idx2", [P, n_tiles * POS // 16], mybir.dt.int16, kind="ExternalInput"
    ).ap()
    signs = nc.dram_tensor(
        "signs", [P, N_CN], mybir.dt.float32, kind="ExternalInput"
    ).ap()
    out = nc.dram_tensor(
        "out", [P, 1], mybir.dt.float32, kind="ExternalOutput"
    ).ap()
    dbg_aps = None
    if dbg:
        dbg_aps = {
            "canon": nc.dram_tensor(
                "canon_dbg",
                [P, n_tiles, N_SLOTS, D],
                mybir.dt.float8e4,
                kind="ExternalOutput",
            ).ap(),
            "scores": nc.dram_tensor(
                "scores_dbg", [P, n_tiles, N_CN], mybir.dt.float32,
                kind="ExternalOutput",
            ).ap(),
        }
    with tile.TileContext(nc) as tc:
        emit_cbow_body(nc, tc, tbl, idx1, idx2, signs, out, n_tiles, dbg=dbg_aps)
    nc.compile()
    return nc


_NC_CACHE = {}


def _get_program():
    if "nc" not in _NC_CACHE:
        _NC_CACHE["nc"] = build_program()
    return _NC_CACHE["nc"]


def make_table(context_weight, center_weight):
    tbl = np.zeros((TBL_ROWS, TBL_W), dtype=ml_dtypes.float8_e4m3fn)
    tbl[:VOCAB, :D] = (np.asarray(context_weight, np.float32) * SCALE).astype(
        ml_dtypes.float8_e4m3fn
    )
    tbl[VOCAB:, :D] = (np.asarray(center_weight, np.float32) * SCALE).astype(
        ml_dtypes.float8_e4m3fn
    )
    return tbl


def pack_gidx(context, center, negatives, batch=BATCH):
    """[batch, N_SLOTS] int32 global table rows: ctx cols 0..9, center col
    10 and negs 11..30 offset by VOCAB."""
    ctx = np.asarray(context, dtype=np.int64).reshape(batch, N_CTX)
    cen = np.asarray(center, dtype=np.int64).reshape(batch, 1) + VOCAB
    neg = np.asarray(negatives, dtype=np.int64).reshape(batch, N_NEG) + VOCAB
    return np.concatenate([ctx, cen, neg], axis=1).astype(np.int32)


def plane(lst):
    """int16 list (len%16==0) -> [128, len/16] plane: position i sits at
    partition i%16 (replicated across the 8 16-partition groups), col
    i//16 -- the layout the gather ucode's index reader expects."""
    arr = np.asarray(lst, dtype=np.int16).reshape(-1, 16).T  # [16, cols]
    return np.tile(arr, (8, 1))


def build_planes(gidx_core, n_tiles=N_TILES):
    """Stage-1/stage-2 index planes for one core.

    gidx_core: [n_tiles*P, N_SLOTS] int32 global table rows.
    Returns (idx1 [P, n_sb*S1_IDX/16], idx2 [P, n_tiles*POS/16]) int16.
    """
    n_sb = n_tiles // TILES_PER_SB
    p1 = np.empty((P, n_sb * S1_IDX // 16), dtype=np.int16)
    p2 = np.empty((P, n_tiles * POS // 16), dtype=np.int16)
    for sb in range(n_sb):
        rows = gidx_core[sb * TILES_PER_SB * P : (sb + 1) * TILES_PER_SB * P]
        # position q = tile_in_sb*POS + slot*128 + p
        pos_gidx = np.concatenate(
            [
                np.ascontiguousarray(rows[ti * P : (ti + 1) * P].T).reshape(-1)
                for ti in range(TILES_PER_SB)
            ]
        )  # [SB_POS]
        chunk = pos_gidx >> 15
        s1_idx = np.zeros(S1_IDX, np.int16)
        s2_slot = np.empty(SB_POS, np.int32)
        filled = [0] * N_CHUNKS  # positions consumed per chunk
        off = 0
        for k, (c, cap) in enumerate(S1_INSTRS):
            sel = np.nonzero(chunk == c)[0][filled[c] : filled[c] + cap]
            filled[c] += sel.size
            s1_idx[off : off + sel.size] = (pos_gidx[sel] & (CHUNK - 1)).astype(
                np.int16
            )
            s2_slot[sel] = S1_RANK0[k] * P + np.arange(sel.size)
            off += cap
        for c in range(N_CHUNKS):
            total = int((chunk == c).sum())
            if filled[c] != total:
                raise RuntimeError(
                    f"chunk {c}: {total} positions exceed caps ({filled[c]})"
                )
        p1[:, sb * (S1_IDX // 16) : (sb + 1) * (S1_IDX // 16)] = plane(s1_idx)
        s2 = s2_slot.astype(np.int16).reshape(TILES_PER_SB, POS)
        for ti in range(TILES_PER_SB):
            t = sb * TILES_PER_SB + ti
            p2[:, t * (POS // 16) : (t + 1) * (POS // 16)] = plane(s2[ti])
    return p1, p2


def make_signs():
    return np.tile(
        np.array([[DESCALE] + [-DESCALE] * N_NEG], dtype=np.float32), (P, 1)
    )


def make_in_maps(context, center, negatives, context_weight, center_weight):
    tbl = make_table(context_weight, center_weight)
    gidx = pack_gidx(context, center, negatives).reshape(N_CORES, B_CORE, N_SLOTS)
    signs = make_signs()
    maps = []
    for c in range(N_CORES):
        idx1, idx2 = build_planes(gidx[c])
        maps.append({"tbl": tbl, "idx1": idx1, "idx2": idx2, "signs": signs})
    return maps


def kernel(context, center, negatives, context_weight, center_weight):
    from concourse import bass_utils

    nc = _get_program()
    in_maps = make_in_maps(
        context, center, negatives, context_weight, center_weight
    )
    res = bass_utils.run_bass_kernel_spmd(nc, in_maps, core_ids=list(range(N_CORES)))
    acc = np.stack([r["out"] for r in res.results])  # [N_CORES, P, N_TILES]
    return np.array(acc.sum(dtype=np.float64) / BATCH, dtype=np.float32)
